# revision 37
# baseline (speedup 1.0000x reference)
# Distributed causal self-attention for 8 Trainium2 NeuronCores.
#
# Problem: B=2, T=2048, C=768, H=12 heads, D=64. y = proj(attn(qkv(x))).
#
# Sharding: 2 (batch) x 4 (head-groups of 3 heads). Core c handles batch
# c//4 and heads (c%4)*3 .. +3. Each core computes its slice of the QKV
# projection, full attention for its 3 heads, and a partial output
# projection y_part = O_heads @ Wp_slice.T. Host sums the 4 partials per
# batch and adds b_proj.
#
# Device-side layout avoids all transposes:
#   qT,kT [d, t]   <- Wqk stationary, xT moving       (per head: 64 rows)
#   sT    [tk, tq] <- k-slice stationary, qT moving   (causal: skip tq < tk_tile)
#   exp   via ScalarE (no max subtraction: |s| <= ~40, exp fits f32)
#   OT+rowsum [65, tq] <- [v | ones] stationary, exp(sT) moving (f32 accum)
#   normalize: ones-matmul broadcast of rowsum, reciprocal_approx_fast, mult
#   y     [t, c2]  <- OT stationary (K=192 over head dims), WpT moving
# Matmul operands bf16, accumulation f32. ScalarE does ONLY exp (it is the
# bottleneck engine); all PSUM evictions run on DVE, causal masks on GPSIMD.
# Attention runs in two tq-passes of 1024 so the OT accumulator takes 2 PSUM
# banks, leaving 6 banks of [128,1024] working tiles.

import numpy as np

B, T, C, H, D = 2, 2048, 768, 12, 64
HPG = 3                      # heads per group
G = 4                        # head groups
CPG = HPG * D                # 192 channels per group
KT = C // 128                # 6 contraction tiles for projections
NT = T // 128                # 16 seq tiles
PW = 1024                    # tq pass width
SCALE = float(1.0 / np.sqrt(2.0))   # 1/sqrt(B) (faithful to reference)

_CACHE = {}


def _build_module():
    import concourse.bass as bass
    import concourse.tile as tile
    import concourse.mybir as mybir
    from concourse.bacc import Bacc
    from contextlib import ExitStack

    f32 = mybir.dt.float32
    bf16 = mybir.dt.bfloat16
    AF = mybir.ActivationFunctionType

    # Bacc (not plain Bass): its compile() legalizes sync waits to the TRN2
    # hardware limit of one wait per instruction (generate_event_semaphores).
    nc = Bacc()

    xt_d = nc.dram_tensor("xt", [C, T], bf16, kind="ExternalInput")
    wqkt_d = nc.dram_tensor("wqkt", [C, 2 * CPG], bf16, kind="ExternalInput")
    wvt_d = nc.dram_tensor("wvt", [C, CPG], bf16, kind="ExternalInput")
    bqk_d = nc.dram_tensor("bqk", [128, 4], f32, kind="ExternalInput")
    bv_d = nc.dram_tensor("bv", [128, CPG], f32, kind="ExternalInput")
    wpt_d = nc.dram_tensor("wpt", [CPG, C], bf16, kind="ExternalInput")
    mask_d = nc.dram_tensor("mask", [128, 128], bf16, kind="ExternalInput")
    # partial outputs in bf16: halves the output DMA tail; host upcasts and
    # sums in f32 (adds ~0.2% relative error, well within budget)
    y_d = nc.dram_tensor("y", [T, C], bf16, kind="ExternalOutput")

    with tile.TileContext(nc) as tc, ExitStack() as ctx:
        sb = ctx.enter_context(tc.tile_pool(name="sb", bufs=1))
        ps = ctx.enter_context(tc.tile_pool(name="ps", bufs=1, space="PSUM"))

        def mm_tile(name):
            return ps.tile([128, 1024], f32, tag="mm", bufs=3, name=name)

        # ---- constants and weights into SBUF ----
        # (wqkt first: the first QKV matmul needs wqkt[0] + xt[0], so the PE
        # starts as early as possible)
        wqkt_sb = []
        for k in range(KT):
            t_ = sb.tile([128, 2 * CPG], bf16, tag=f"wqk{k}", name=f"wqk{k}")
            nc.sync.dma_start(t_[:, :], wqkt_d[k * 128:(k + 1) * 128, :])
            wqkt_sb.append(t_)
        # xt DMA'd in 512-column chunks so the first QKV matmul chain can
        # start after ~1.4MB instead of ~3.6MB of DMA
        xt_sb = []
        for k in range(KT):
            t_ = sb.tile([128, T], bf16, tag=f"xt{k}", name=f"xt{k}")
            xt_sb.append(t_)
        # first two chunk-columns issued from the Vector engine's DMA queue:
        # the Sync sequencer takes ~650ns per DMA_DIRECT2D issue, and these
        # 12 transfers gate the first matmul
        for c in range(0, T, 512):
            eng = nc.scalar if c < 1024 else nc.sync
            for k in range(KT):
                eng.dma_start(xt_sb[k][:, c:c + 512],
                              xt_d[k * 128:(k + 1) * 128, c:c + 512])
        wvt_sb = []
        for k in range(KT):
            t_ = sb.tile([128, CPG], bf16, tag=f"wv{k}", name=f"wv{k}")
            nc.sync.dma_start(t_[:, :], wvt_d[k * 128:(k + 1) * 128, :])
            wvt_sb.append(t_)
        bqk_sb = sb.tile([128, 4], f32, tag="bqk", name="bqk")
        nc.sync.dma_start(bqk_sb[:, :], bqk_d[:, :])
        bv_sb = sb.tile([128, CPG], f32, tag="bv", name="bv")
        nc.sync.dma_start(bv_sb[:, :], bv_d[:, :])
        mask_sb = sb.tile([128, 128], bf16, tag="mask", name="mask")
        nc.sync.dma_start(mask_sb[:, :], mask_d[:, :])
        wpt0_sb = sb.tile([128, C], bf16, tag="wpt0", name="wpt0")
        nc.sync.dma_start(wpt0_sb[:, :], wpt_d[0:128, :])
        # K-pad the second proj K-tile to 128 rows (zeros) so the matmul
        # drives the full PE array: HAM clock-gates the PE to half speed when
        # the array is partially active, and zero rows cost no extra cycles.
        wpt1_sb = sb.tile([128, C], bf16, tag="wpt1", name="wpt1")
        nc.gpsimd.memset(wpt1_sb[64:128, :], 0.0)
        nc.sync.dma_start(wpt1_sb[0:64, :], wpt_d[128:CPG, :])
        ones_sb = sb.tile([1, 128], bf16, tag="ones", name="ones")
        nc.vector.memset(ones_sb[:, :], 1.0)
        # Warm the ScalarE exp spline table at kernel start: the implicit
        # ACT_TABLE_LOAD (~2.7us) would otherwise stall the PE right when
        # attention begins, dropping the HAM clock gate to 4/8 for the rest
        # of the run.
        expwarm = sb.tile([1, 128], f32, tag="expwarm", name="expwarm")
        nc.scalar.activation(expwarm[:, :], ones_sb[:, :], AF.Exp)

        # ---- QKV projection (q,k transposed; v natural) ----
        # Per-head qT/kT tiles, K-padded to 128 partitions with zero rows
        # 64:128 so attention S-matmuls drive the full PE array (see above).
        qh_sb, kh_sb = [], []
        for h in range(HPG):
            qh = sb.tile([128, T], bf16, tag=f"qh{h}", name=f"qh{h}")
            nc.gpsimd.memset(qh[64:128, :], 0.0)
            qh_sb.append(qh)
            kh = sb.tile([128, T], bf16, tag=f"kh{h}", name=f"kh{h}")
            nc.gpsimd.memset(kh[64:128, :], 0.0)
            kh_sb.append(kh)
        # qkv matmul M-tiles stay packed: [q0 q1 | q2 | k0 k1 | k2].
        # Emission order puts head-0/1 q,k first so attention starts earliest.
        mtiles = [(0, 128, 0, [qh_sb[0], qh_sb[1]]),
                  (2, 128, 192, [kh_sb[0], kh_sb[1]]),
                  (1, 64, 128, [qh_sb[2]]),
                  (3, 64, 320, [kh_sb[2]])]
        def qk_unit(m, rows, c0, dsts, c):
            pq = mm_tile(f"pq{m}_{c}")
            for b0 in (0, 512):
                for k in range(KT):
                    nc.tensor.matmul(
                        pq[0:rows, b0:b0 + 512],
                        lhsT=wqkt_sb[k][:, c0:c0 + rows],
                        rhs=xt_sb[k][:, c + b0:c + b0 + 512],
                        start=(k == 0), stop=(k == KT - 1),
                    )
            for di, dst in enumerate(dsts):
                nc.vector.tensor_scalar_add(
                    dst[0:64, c:c + 1024], pq[di * 64:di * 64 + 64, :],
                    bqk_sb[di * 64:di * 64 + 64, m:m + 1])

        v_sb = [None] * NT

        def v_unit(t):
            pv = mm_tile(f"pv{t}")
            for k in range(KT):
                nc.tensor.matmul(
                    pv[:, 0:CPG],
                    lhsT=xt_sb[k][:, t * 128:(t + 1) * 128],
                    rhs=wvt_sb[k][:, :],
                    start=(k == 0), stop=(k == KT - 1),
                )
            # per head: [v (64) | ones (1) | zeros (63)] -> O-matmul lhsT is a
            # full 128-column stationary operand (M-pad, keeps PE array full)
            vt = sb.tile([128, HPG * 128], bf16, tag=f"v{t}", name=f"v{t}")
            vt3 = vt.rearrange("p (h u) -> p h u", u=128)
            nc.gpsimd.memset(vt3[:, :, 65:128], 0.0)
            nc.vector.memset(vt3[:, :, 64:65], 1.0)
            nc.vector.tensor_add(
                vt3[:, :, 0:64],
                pv[:, 0:CPG].rearrange("p (h d) -> p h d", d=64),
                bv_sb[:, :].rearrange("p (h d) -> p h d", d=64),
            )
            v_sb[t] = vt

        # interleave v-projection units between qkv units so the DVE eviction
        # backlog never leaves the PE without ready matmul work
        qk_units = [(m, rows, c0, dsts, c)
                    for m, rows, c0, dsts in mtiles for c in (0, 1024)]
        for j, qu in enumerate(qk_units):
            qk_unit(*qu)
            if j >= 4:
                for t in range((j - 4) * 4, (j - 3) * 4):
                    v_unit(t)

        # ---- attention: heads sequential, each in two tq-passes of 1024 ----
        pt0 = sb.tile([128, T], bf16, tag="pt0", name="pt0")
        pt1 = sb.tile([128, T], bf16, tag="pt1", name="pt1")
        nc.gpsimd.memset(pt1[64:128, :], 0.0)
        p_slices = [(pt0, 0), (pt0, 64), (pt1, 0)]

        ex_store = {}
        ot_store = {}

        def make_units(h, p):
            """Return (s_units, o_units) closures for one (head, pass)."""
            qv = qh_sb[h]
            kv = kh_sb[h]
            pdst, po = p_slices[h]
            base = p * PW
            i_max = (base + PW) // 128

            def get_ot():
                # allocated lazily at the first O matmul of this pass so the
                # single ot slot (bufs=1) is handed over as late as possible
                if (h, p) not in ot_store:
                    ot_store[(h, p)] = ps.tile([128, PW], f32, tag="ot",
                                               bufs=1, name=f"ot{h}_{p}")
                return ot_store[(h, p)]

            def emit_s(i):
                lo = max(i * 128, base)
                ex = sb.tile([128, PW], bf16, tag="ex", bufs=5,
                             name=f"ex{h}_{p}_{i}")
                sp = mm_tile(f"sp{h}_{p}_{i}")
                for b0 in (0, 512):
                    cs = max(lo, base + b0)
                    ce = base + b0 + 512
                    if cs >= ce:
                        continue
                    nc.tensor.matmul(
                        sp[:, cs - base:ce - base],
                        lhsT=kv[:, i * 128:(i + 1) * 128],
                        rhs=qv[:, cs:ce],
                        start=True, stop=True,
                    )
                nc.scalar.activation(ex[:, lo - base:PW], sp[:, lo - base:PW],
                                     AF.Exp, scale=SCALE)
                if lo == i * 128:
                    # causal mask of the diagonal 128x128 block (DVE; GPSIMD
                    # is reserved for the attn-library partition_broadcast)
                    r = i * 128 - base
                    nc.vector.tensor_mul(
                        ex[:, r:r + 128], ex[:, r:r + 128], mask_sb[:, :])
                ex_store[(h, p, i)] = ex

            def norm_chunk(b0):
                # PE-free normalization: rowsum copy + reciprocal on DVE,
                # partition broadcast on GPSIMD (attn ucode library, loaded
                # once by Bacc), final scale on DVE.
                ot = get_ot()
                rsb = sb.tile([1, 512], f32, tag="rsb", bufs=4,
                              name=f"rsb{h}_{p}_{b0}")
                nc.vector.tensor_copy(rsb[:, :], ot[64:65, b0:b0 + 512])
                rcp = sb.tile([1, 512], f32, tag="rcp", bufs=4,
                              name=f"rcp{h}_{p}_{b0}")
                nc.vector.reciprocal_approx_fast(rcp[:, :], rsb[:, :])
                rb = sb.tile([64, 512], f32, tag="rb", bufs=2,
                             name=f"rb{h}_{p}_{b0}")
                nc.gpsimd.partition_broadcast(rb[:, :], rcp[:, :])
                nc.vector.tensor_mul(
                    pdst[po:po + 64, base + b0:base + b0 + 512],
                    ot[0:64, b0:b0 + 512], rb[:, :])

            def emit_o(i):
                lo = max(i * 128, base)
                ex = ex_store.pop((h, p, i))
                ot = get_ot()
                for b0 in (0, 512):
                    cs = max(lo, base + b0)
                    ce = base + b0 + 512
                    if cs >= ce:
                        continue
                    last_i = min(i_max - 1, (base + b0) // 128 + 3)
                    nc.tensor.matmul(
                        ot[:, cs - base:ce - base],
                        lhsT=v_sb[i][:, h * 128:(h + 1) * 128],
                        rhs=ex[:, cs - base:ce - base],
                        start=(i == 0), stop=(i == last_i),
                    )
                    if i == last_i:
                        norm_chunk(b0)

            s_units = [lambda i=i: emit_s(i) for i in range(i_max)]
            o_units = [lambda i=i: emit_o(i) for i in range(i_max)]
            return s_units, o_units

        def emit_proj_tile(t):
            yp = mm_tile(f"yp{t}")
            for n0, nn in ((0, 512), (512, 256)):
                nc.tensor.matmul(yp[:, n0:n0 + nn],
                                 lhsT=pt0[:, t * 128:(t + 1) * 128],
                                 rhs=wpt0_sb[:, n0:n0 + nn],
                                 start=True, stop=False)
                nc.tensor.matmul(yp[:, n0:n0 + nn],
                                 lhsT=pt1[:, t * 128:(t + 1) * 128],
                                 rhs=wpt1_sb[:, n0:n0 + nn],
                                 start=False, stop=True)
            ysb = sb.tile([128, C], bf16, tag=f"ysb{t % 4}", bufs=2,
                          name=f"ysb{t}")
            nc.vector.tensor_copy(ysb[:, 0:C], yp[:, 0:C])
            nc.sync.dma_start(y_d[t * 128:(t + 1) * 128, :], ysb[:, 0:C])

        # Flat software pipeline across all (head, pass) boundaries: the
        # S stream runs `depth` units ahead of the O stream so the PE never
        # drains at a pass boundary; proj tiles fill the final O tail.
        all_s, all_o = [], []
        for h in range(HPG):
            for p in range(2):
                s_u, o_u = make_units(h, p)
                all_s += s_u
                all_o += o_u
        depth = 4
        for idx, s_unit in enumerate(all_s):
            s_unit()
            if idx >= depth:
                all_o[idx - depth]()
        tail_o = all_o[len(all_s) - depth:]
        for j, o_unit in enumerate(tail_o):
            o_unit()
            for t in range(j * 4, min((j + 1) * 4, NT)):
                emit_proj_tile(t)
        for t in range(len(tail_o) * 4, NT):
            emit_proj_tile(t)

    nc.finalize()
    return nc


def _get_module():
    if "nc" not in _CACHE:
        _CACHE["nc"] = _build_module()
    return _CACHE["nc"]


def make_in_maps(x, w_attn, b_attn, w_proj):
    """Host-side sharding: per-core input dicts (8 cores)."""
    import ml_dtypes
    bf16 = ml_dtypes.bfloat16
    x = np.asarray(x, dtype=np.float32)
    w_attn = np.asarray(w_attn, dtype=np.float32)
    b_attn = np.asarray(b_attn, dtype=np.float32)
    w_proj = np.asarray(w_proj, dtype=np.float32)

    xts = [np.ascontiguousarray(x[b].T).astype(bf16) for b in range(B)]
    mask = np.triu(np.ones((128, 128), np.float32)).astype(bf16)

    in_maps = []
    for c in range(8):
        b = c // G
        hg = c % G
        sl = slice(CPG * hg, CPG * (hg + 1))
        wq = w_attn[0:C][sl]
        wk = w_attn[C:2 * C][sl]
        wv = w_attn[2 * C:3 * C][sl]
        wqkt = np.ascontiguousarray(
            np.concatenate([wq, wk], axis=0).T).astype(bf16)      # [768, 384]
        wvt = np.ascontiguousarray(wv.T).astype(bf16)             # [768, 192]
        bq = b_attn[0:C][sl]
        bk = b_attn[C:2 * C][sl]
        bv = b_attn[2 * C:3 * C][sl]
        bqk = np.zeros((128, 4), np.float32)
        bqk[:, 0] = bq[0:128]
        bqk[0:64, 1] = bq[128:192]
        bqk[:, 2] = bk[0:128]
        bqk[0:64, 3] = bk[128:192]
        bvb = np.ascontiguousarray(
            np.broadcast_to(bv, (128, CPG))).astype(np.float32)   # [128, 192]
        wpt = np.ascontiguousarray(w_proj[:, sl].T).astype(bf16)  # [192, 768]
        in_maps.append({
            "xt": xts[b],
            "wqkt": wqkt,
            "wvt": wvt,
            "bqk": bqk,
            "bv": bvb,
            "wpt": wpt,
            "mask": mask,
        })
    return in_maps


def gather(results, b_proj):
    """Sum the 4 head-group partials per batch, add bias."""
    b_proj = np.asarray(b_proj, dtype=np.float32)
    y = np.zeros((B, T, C), np.float32)
    for c in range(8):
        y[c // G] += np.asarray(results[c]["y"], dtype=np.float32)
    y += b_proj
    return y


def run(x, w_attn, b_attn, w_proj, b_proj, trace=False, **kw):
    from concourse.bass_utils import run_bass_kernel_spmd
    nc = _get_module()
    in_maps = make_in_maps(x, w_attn, b_attn, w_proj)
    res = run_bass_kernel_spmd(nc, in_maps, list(range(8)), trace=trace, **kw)
    return gather(res.results, b_proj), res


def kernel(x, w_attn, b_attn, w_proj, b_proj):
    y, _ = run(x, w_attn, b_attn, w_proj, b_proj)
    return y


# revision 42
# speedup vs baseline: 1.0860x; 1.0860x over previous
# Distributed causal self-attention for 8 Trainium2 NeuronCores.
#
# Problem: B=2, T=2048, C=768, H=12 heads, D=64. y = proj(attn(qkv(x))).
#
# Sharding: 2 (batch) x 4 (head-groups of 3 heads). Core c handles batch
# c//4 and heads (c%4)*3 .. +3. Each core computes its slice of the QKV
# projection, full attention for its 3 heads, and a partial output
# projection y_part = O_heads @ Wp_slice.T. Host sums the 4 partials per
# batch and adds b_proj.
#
# Device-side layout avoids all transposes:
#   qT,kT [d, t]   <- Wqk stationary, xT moving       (per head: 64 rows)
#   sT    [tk, tq] <- k-slice stationary, qT moving   (causal: skip tq < tk_tile)
#   exp   via ScalarE (no max subtraction: |s| <= ~40, exp fits f32)
#   OT+rowsum [65, tq] <- [v | ones] stationary, exp(sT) moving (f32 accum)
#   normalize: ones-matmul broadcast of rowsum, reciprocal_approx_fast, mult
#   y     [t, c2]  <- OT stationary (K=192 over head dims), WpT moving
# Matmul operands bf16, accumulation f32. ScalarE does ONLY exp (it is the
# bottleneck engine); all PSUM evictions run on DVE, causal masks on GPSIMD.
# Attention runs in two tq-passes of 1024 so the OT accumulator takes 2 PSUM
# banks, leaving 6 banks of [128,1024] working tiles.

import numpy as np

B, T, C, H, D = 2, 2048, 768, 12, 64
HPG = 3                      # heads per group
G = 4                        # head groups
CPG = HPG * D                # 192 channels per group
KT = C // 128                # 6 contraction tiles for projections
NT = T // 128                # 16 seq tiles
PW = 1024                    # tq pass width
SCALE = float(1.0 / np.sqrt(2.0))   # 1/sqrt(B) (faithful to reference)

_CACHE = {}


def _build_module():
    import concourse.bass as bass
    import concourse.tile as tile
    import concourse.mybir as mybir
    from concourse.bacc import Bacc
    from contextlib import ExitStack

    f32 = mybir.dt.float32
    bf16 = mybir.dt.bfloat16
    AF = mybir.ActivationFunctionType

    # Bacc (not plain Bass): its compile() legalizes sync waits to the TRN2
    # hardware limit of one wait per instruction (generate_event_semaphores).
    nc = Bacc()

    xt_d = nc.dram_tensor("xt", [C, T], bf16, kind="ExternalInput")
    wqkt_d = nc.dram_tensor("wqkt", [C, 2 * CPG], bf16, kind="ExternalInput")
    wvt_d = nc.dram_tensor("wvt", [C, CPG], bf16, kind="ExternalInput")
    bqk_d = nc.dram_tensor("bqk", [128, 4], f32, kind="ExternalInput")
    bv_d = nc.dram_tensor("bv", [128, CPG], f32, kind="ExternalInput")
    wpt_d = nc.dram_tensor("wpt", [CPG, C], bf16, kind="ExternalInput")
    mask_d = nc.dram_tensor("mask", [128, 128], bf16, kind="ExternalInput")
    # partial outputs in bf16: halves the output DMA tail; host upcasts and
    # sums in f32 (adds ~0.2% relative error, well within budget)
    y_d = nc.dram_tensor("y", [T, C], bf16, kind="ExternalOutput")

    with tile.TileContext(nc) as tc, ExitStack() as ctx:
        sb = ctx.enter_context(tc.tile_pool(name="sb", bufs=1))
        ps = ctx.enter_context(tc.tile_pool(name="ps", bufs=1, space="PSUM"))

        def mm_tile(name):
            return ps.tile([128, 1024], f32, tag="mm", bufs=3, name=name)

        # ---- constants and weights into SBUF ----
        # (wqkt first: the first QKV matmul needs wqkt[0] + xt[0], so the PE
        # starts as early as possible)
        wqkt_sb = []
        for k in range(KT):
            t_ = sb.tile([128, 2 * CPG], bf16, tag=f"wqk{k}", name=f"wqk{k}")
            nc.sync.dma_start(t_[:, :], wqkt_d[k * 128:(k + 1) * 128, :])
            wqkt_sb.append(t_)
        # xt DMA'd in 512-column chunks so the first QKV matmul chain can
        # start after ~1.4MB instead of ~3.6MB of DMA
        xt_sb = []
        for k in range(KT):
            t_ = sb.tile([128, T], bf16, tag=f"xt{k}", name=f"xt{k}")
            xt_sb.append(t_)
        # first two chunk-columns issued from the Vector engine's DMA queue:
        # the Sync sequencer takes ~650ns per DMA_DIRECT2D issue, and these
        # 12 transfers gate the first matmul
        for c in range(0, T, 512):
            eng = nc.scalar if c < 1024 else nc.sync
            for k in range(KT):
                eng.dma_start(xt_sb[k][:, c:c + 512],
                              xt_d[k * 128:(k + 1) * 128, c:c + 512])
        wvt_sb = []
        for k in range(KT):
            t_ = sb.tile([128, CPG], bf16, tag=f"wv{k}", name=f"wv{k}")
            nc.sync.dma_start(t_[:, :], wvt_d[k * 128:(k + 1) * 128, :])
            wvt_sb.append(t_)
        bqk_sb = sb.tile([128, 4], f32, tag="bqk", name="bqk")
        nc.sync.dma_start(bqk_sb[:, :], bqk_d[:, :])
        bv_sb = sb.tile([128, CPG], f32, tag="bv", name="bv")
        nc.sync.dma_start(bv_sb[:, :], bv_d[:, :])
        mask_sb = sb.tile([128, 128], bf16, tag="mask", name="mask")
        nc.sync.dma_start(mask_sb[:, :], mask_d[:, :])
        wpt0_sb = sb.tile([128, C], bf16, tag="wpt0", name="wpt0")
        nc.sync.dma_start(wpt0_sb[:, :], wpt_d[0:128, :])
        # K-pad the second proj K-tile to 128 rows (zeros) so the matmul
        # drives the full PE array: HAM clock-gates the PE to half speed when
        # the array is partially active, and zero rows cost no extra cycles.
        wpt1_sb = sb.tile([128, C], bf16, tag="wpt1", name="wpt1")
        nc.gpsimd.memset(wpt1_sb[64:128, :], 0.0)
        nc.sync.dma_start(wpt1_sb[0:64, :], wpt_d[128:CPG, :])
        ones_sb = sb.tile([1, 128], bf16, tag="ones", name="ones")
        nc.vector.memset(ones_sb[:, :], 1.0)
        # Warm the ScalarE exp spline table at kernel start: the implicit
        # ACT_TABLE_LOAD (~2.7us) would otherwise stall the PE right when
        # attention begins, dropping the HAM clock gate to 4/8 for the rest
        # of the run.
        expwarm = sb.tile([1, 128], f32, tag="expwarm", name="expwarm")
        nc.scalar.activation(expwarm[:, :], ones_sb[:, :], AF.Exp)

        # ---- QKV projection (q,k transposed; v natural) ----
        # Per-head qT/kT tiles, K-padded to 128 partitions with zero rows
        # 64:128 so attention S-matmuls drive the full PE array (see above).
        qh_sb, kh_sb = [], []
        for h in range(HPG):
            qh = sb.tile([128, T], bf16, tag=f"qh{h}", name=f"qh{h}")
            nc.gpsimd.memset(qh[64:128, :], 0.0)
            qh_sb.append(qh)
            kh = sb.tile([128, T], bf16, tag=f"kh{h}", name=f"kh{h}")
            nc.gpsimd.memset(kh[64:128, :], 0.0)
            kh_sb.append(kh)
        # qkv matmul M-tiles stay packed: [q0 q1 | q2 | k0 k1 | k2].
        # Emission order puts head-0/1 q,k first so attention starts earliest.
        mtiles = [(0, 128, 0, [qh_sb[0], qh_sb[1]]),
                  (2, 128, 192, [kh_sb[0], kh_sb[1]]),
                  (1, 64, 128, [qh_sb[2]]),
                  (3, 64, 320, [kh_sb[2]])]
        def qk_unit(m, rows, c0, dsts, c):
            pq = mm_tile(f"pq{m}_{c}")
            for b0 in (0, 512):
                for k in range(KT):
                    nc.tensor.matmul(
                        pq[0:rows, b0:b0 + 512],
                        lhsT=wqkt_sb[k][:, c0:c0 + rows],
                        rhs=xt_sb[k][:, c + b0:c + b0 + 512],
                        start=(k == 0), stop=(k == KT - 1),
                    )
            for di, dst in enumerate(dsts):
                # alternate evictions between DVE and ScalarE (idle during
                # qkv): one engine alone becomes a 1.3us-per-tile backlog
                if di == 0:
                    nc.vector.tensor_scalar_add(
                        dst[0:64, c:c + 1024], pq[di * 64:di * 64 + 64, :],
                        bqk_sb[di * 64:di * 64 + 64, m:m + 1])
                else:
                    nc.scalar.activation(
                        dst[0:64, c:c + 1024], pq[di * 64:di * 64 + 64, :],
                        AF.Identity, bias=bqk_sb[di * 64:di * 64 + 64, m:m + 1])

        v_sb = [None] * NT

        def v_unit(t):
            pv = mm_tile(f"pv{t}")
            for k in range(KT):
                nc.tensor.matmul(
                    pv[:, 0:CPG],
                    lhsT=xt_sb[k][:, t * 128:(t + 1) * 128],
                    rhs=wvt_sb[k][:, :],
                    start=(k == 0), stop=(k == KT - 1),
                )
            # per head: [v (64) | ones (1) | zeros (63)] -> O-matmul lhsT is a
            # full 128-column stationary operand (M-pad, keeps PE array full)
            vt = sb.tile([128, HPG * 128], bf16, tag=f"v{t}", name=f"v{t}")
            vt3 = vt.rearrange("p (h u) -> p h u", u=128)
            nc.gpsimd.memset(vt3[:, :, 65:128], 0.0)
            nc.vector.memset(vt3[:, :, 64:65], 1.0)
            nc.vector.tensor_add(
                vt3[:, :, 0:64],
                pv[:, 0:CPG].rearrange("p (h d) -> p h d", d=64),
                bv_sb[:, :].rearrange("p (h d) -> p h d", d=64),
            )
            v_sb[t] = vt

        # interleave v-projection units between qkv units so the DVE eviction
        # backlog never leaves the PE without ready matmul work
        qk_units = [(m, rows, c0, dsts, c)
                    for m, rows, c0, dsts in mtiles for c in (0, 1024)]
        for j, qu in enumerate(qk_units):
            qk_unit(*qu)
            if j >= 4:
                for t in range((j - 4) * 4, (j - 3) * 4):
                    v_unit(t)

        # ---- attention: heads sequential, each in two tq-passes of 1024 ----
        pt0 = sb.tile([128, T], bf16, tag="pt0", name="pt0")
        pt1 = sb.tile([128, T], bf16, tag="pt1", name="pt1")
        nc.gpsimd.memset(pt1[64:128, :], 0.0)
        p_slices = [(pt0, 0), (pt0, 64), (pt1, 0)]

        ex_store = {}
        ot_store = {}
        pending_norm = []  # deferred second halves of norm chains

        def make_units(h, p):
            """Return (s_units, o_units) closures for one (head, pass)."""
            qv = qh_sb[h]
            kv = kh_sb[h]
            pdst, po = p_slices[h]
            base = p * PW
            i_max = (base + PW) // 128

            def get_ot():
                # allocated lazily at the first O matmul of this pass so the
                # single ot slot (bufs=1) is handed over as late as possible
                if (h, p) not in ot_store:
                    ot_store[(h, p)] = ps.tile([128, PW], f32, tag="ot",
                                               bufs=1, name=f"ot{h}_{p}")
                return ot_store[(h, p)]

            def emit_s(i):
                lo = max(i * 128, base)
                ex = sb.tile([128, PW], bf16, tag="ex", bufs=5,
                             name=f"ex{h}_{p}_{i}")
                sp = mm_tile(f"sp{h}_{p}_{i}")
                for b0 in (0, 512):
                    cs = max(lo, base + b0)
                    ce = base + b0 + 512
                    if cs >= ce:
                        continue
                    nc.tensor.matmul(
                        sp[:, cs - base:ce - base],
                        lhsT=kv[:, i * 128:(i + 1) * 128],
                        rhs=qv[:, cs:ce],
                        start=True, stop=True,
                    )
                nc.scalar.activation(ex[:, lo - base:PW], sp[:, lo - base:PW],
                                     AF.Exp, scale=SCALE)
                if lo == i * 128:
                    # causal mask of the diagonal 128x128 block (GPSIMD: DVE
                    # and ScalarE are the loaded engines)
                    r = i * 128 - base
                    nc.gpsimd.tensor_mul(
                        ex[:, r:r + 128], ex[:, r:r + 128], mask_sb[:, :])
                ex_store[(h, p, i)] = ex

            def norm_chunk(b0):
                # first half now (rowsum copy on DVE); the PE-visible second
                # half (bs matmul waiting on that copy) is deferred one
                # pipeline unit so the PE never waits on the DVE
                ot = get_ot()
                rsb = sb.tile([1, 512], bf16, tag="rsb", bufs=4,
                              name=f"rsb{h}_{p}_{b0}")
                nc.vector.tensor_copy(rsb[:, :], ot[64:65, b0:b0 + 512])

                def norm_b():
                    bs = mm_tile(f"bs{h}_{p}_{b0}")
                    nc.tensor.matmul(bs[:, 0:512], lhsT=ones_sb[:, :],
                                     rhs=rsb[:, :], start=True, stop=True)
                    rb = sb.tile([64, 512], f32, tag="rb", bufs=2,
                                 name=f"rb{h}_{p}_{b0}")
                    nc.vector.reciprocal_approx_fast(rb[:, :], bs[0:64, 0:512])
                    nc.vector.tensor_mul(
                        pdst[po:po + 64, base + b0:base + b0 + 512],
                        ot[0:64, b0:b0 + 512], rb[:, :])
                pending_norm.append(norm_b)

            def emit_o(i):
                lo = max(i * 128, base)
                ex = ex_store.pop((h, p, i))
                ot = get_ot()
                for b0 in (0, 512):
                    cs = max(lo, base + b0)
                    ce = base + b0 + 512
                    if cs >= ce:
                        continue
                    last_i = min(i_max - 1, (base + b0) // 128 + 3)
                    nc.tensor.matmul(
                        ot[:, cs - base:ce - base],
                        lhsT=v_sb[i][:, h * 128:(h + 1) * 128],
                        rhs=ex[:, cs - base:ce - base],
                        start=(i == 0), stop=(i == last_i),
                    )
                    if i == last_i:
                        norm_chunk(b0)

            s_units = [lambda i=i: emit_s(i) for i in range(i_max)]
            o_units = [lambda i=i: emit_o(i) for i in range(i_max)]
            return s_units, o_units

        def emit_proj_tile(t):
            yp = mm_tile(f"yp{t}")
            for n0, nn in ((0, 512), (512, 256)):
                nc.tensor.matmul(yp[:, n0:n0 + nn],
                                 lhsT=pt0[:, t * 128:(t + 1) * 128],
                                 rhs=wpt0_sb[:, n0:n0 + nn],
                                 start=True, stop=False)
                nc.tensor.matmul(yp[:, n0:n0 + nn],
                                 lhsT=pt1[:, t * 128:(t + 1) * 128],
                                 rhs=wpt1_sb[:, n0:n0 + nn],
                                 start=False, stop=True)
            ysb = sb.tile([128, C], bf16, tag=f"ysb{t % 4}", bufs=2,
                          name=f"ysb{t}")
            nc.vector.tensor_copy(ysb[:, 0:C], yp[:, 0:C])
            nc.sync.dma_start(y_d[t * 128:(t + 1) * 128, :], ysb[:, 0:C])

        # Flat software pipeline across all (head, pass) boundaries: the
        # S stream runs `depth` units ahead of the O stream so the PE never
        # drains at a pass boundary; proj tiles fill the final O tail.
        def drain_norms():
            while pending_norm:
                pending_norm.pop(0)()

        all_s, all_o = [], []
        for h in range(HPG):
            for p in range(2):
                s_u, o_u = make_units(h, p)
                all_s += s_u
                all_o += o_u
        depth = 4
        for idx, s_unit in enumerate(all_s):
            drain_norms()
            s_unit()
            if idx >= depth:
                all_o[idx - depth]()
        tail_o = all_o[len(all_s) - depth:]
        for j, o_unit in enumerate(tail_o):
            o_unit()
            drain_norms()
            for t in range(j * 4, min((j + 1) * 4, NT)):
                emit_proj_tile(t)
        drain_norms()
        for t in range(len(tail_o) * 4, NT):
            emit_proj_tile(t)

    nc.finalize()
    return nc


def _get_module():
    if "nc" not in _CACHE:
        _CACHE["nc"] = _build_module()
    return _CACHE["nc"]


def make_in_maps(x, w_attn, b_attn, w_proj):
    """Host-side sharding: per-core input dicts (8 cores)."""
    import ml_dtypes
    bf16 = ml_dtypes.bfloat16
    x = np.asarray(x, dtype=np.float32)
    w_attn = np.asarray(w_attn, dtype=np.float32)
    b_attn = np.asarray(b_attn, dtype=np.float32)
    w_proj = np.asarray(w_proj, dtype=np.float32)

    xts = [np.ascontiguousarray(x[b].T).astype(bf16) for b in range(B)]
    mask = np.triu(np.ones((128, 128), np.float32)).astype(bf16)

    in_maps = []
    for c in range(8):
        b = c // G
        hg = c % G
        sl = slice(CPG * hg, CPG * (hg + 1))
        wq = w_attn[0:C][sl]
        wk = w_attn[C:2 * C][sl]
        wv = w_attn[2 * C:3 * C][sl]
        wqkt = np.ascontiguousarray(
            np.concatenate([wq, wk], axis=0).T).astype(bf16)      # [768, 384]
        wvt = np.ascontiguousarray(wv.T).astype(bf16)             # [768, 192]
        bq = b_attn[0:C][sl]
        bk = b_attn[C:2 * C][sl]
        bv = b_attn[2 * C:3 * C][sl]
        bqk = np.zeros((128, 4), np.float32)
        bqk[:, 0] = bq[0:128]
        bqk[0:64, 1] = bq[128:192]
        bqk[:, 2] = bk[0:128]
        bqk[0:64, 3] = bk[128:192]
        bvb = np.ascontiguousarray(
            np.broadcast_to(bv, (128, CPG))).astype(np.float32)   # [128, 192]
        wpt = np.ascontiguousarray(w_proj[:, sl].T).astype(bf16)  # [192, 768]
        in_maps.append({
            "xt": xts[b],
            "wqkt": wqkt,
            "wvt": wvt,
            "bqk": bqk,
            "bv": bvb,
            "wpt": wpt,
            "mask": mask,
        })
    return in_maps


def gather(results, b_proj):
    """Sum the 4 head-group partials per batch, add bias."""
    b_proj = np.asarray(b_proj, dtype=np.float32)
    y = np.zeros((B, T, C), np.float32)
    for c in range(8):
        y[c // G] += np.asarray(results[c]["y"], dtype=np.float32)
    y += b_proj
    return y


def run(x, w_attn, b_attn, w_proj, b_proj, trace=False, **kw):
    from concourse.bass_utils import run_bass_kernel_spmd
    nc = _get_module()
    in_maps = make_in_maps(x, w_attn, b_attn, w_proj)
    res = run_bass_kernel_spmd(nc, in_maps, list(range(8)), trace=trace, **kw)
    return gather(res.results, b_proj), res


def kernel(x, w_attn, b_attn, w_proj, b_proj):
    y, _ = run(x, w_attn, b_attn, w_proj, b_proj)
    return y
